# revision 16
# baseline (speedup 1.0000x reference)
"""Trainium2 Bass kernel for nn_Adjacency (dense_mlp).

Reference computation:
    pr = product @ w1[:S]                # [P, S]
    pe = person  @ w1[S:]                # [Q, S]
    h  = softplus(pr[:,None,:] + pe[None,:,:])   # [P, Q, S]
    m  = einsum('pqs,so->pq', h, w2)
    adj = leaky_relu(m, 0.1)
    out = adj[None] * x                  # [B, P, Q]

Sharding: P across 8 cores (128 rows each); person/w1/w2 replicated;
x / out sharded on dim 1. No collectives.

Per-core algorithm (all tiles [partition, free]):
  - pr_T = w1a^T-matmul -> PSUM [s=128, p=128];  Epr = exp(pr_T)  (ACT)
  - pe_T = w1b-matmul   -> PSUM [s=128, q=1024]; Epe = exp(pe_T)  (ACT)
  - for p in 0..127:  h_p[s, q] = ln(Epe * Epr[:, p] + 1)   <- one ACT
    instruction per p: softplus(pr+pe) = ln(1 + exp(pr)exp(pe)); the
    per-partition `scale` AP fuses the broadcast multiply.
  - S-reduction on TensorE: m[p, :] = w2^T @ h_p via M=32 matmuls with
    "column-embedded" w2 (lhsT_j has w2 in column j, zeros elsewhere);
    32 accumulating matmuls build a dense [32, 512] PSUM tile, and the
    four p-groups land on the four 32-aligned PSUM col-groups.
  - leaky-relu on DVE (one scalar_tensor_tensor per group), then
    out[b] = adj * x[b] on DVE, DMA out.
"""

import numpy as np

P, Q, S, B = 1024, 1024, 128, 8
N_CORES = 8
PS = P // N_CORES  # 128 p rows per core
GROUPS = 4         # p-groups of 32 (PSUM col-groups)
GW = PS // GROUPS  # 32


_CACHE = {}


def _build_nc():
    import concourse.bass as bass
    import concourse.tile as tile
    from concourse import mybir

    f32 = mybir.dt.float32
    bf16 = mybir.dt.bfloat16
    AF = mybir.ActivationFunctionType
    ALU = mybir.AluOpType

    nc = bass.Bass()

    # One packed weight blob -> one DMA -> one semaphore (walrus caps the
    # number of sync waits per instruction). Layout per partition (f32):
    # [product_t 128 | person_t 1024 | w1a 128 | w1b 128 | w2emb-bf16 512]
    WTS = PS + Q + S + S + (GW * GW) // 2
    wts = nc.declare_dram_parameter("wts", [S, WTS], f32, isOutput=False)
    x_in = nc.declare_dram_parameter("x", [B, PS, Q], f32, isOutput=False)
    out_e = nc.declare_dram_parameter("out", [B, PS, Q], f32, isOutput=True)

    with tile.TileContext(nc) as tc:
        with (
            tc.tile_pool(name="const", bufs=1) as const,
            tc.tile_pool(name="xbuf", bufs=1) as xbuf,
            tc.tile_pool(name="hbuf", bufs=4) as hbuf,
            tc.tile_pool(name="pa", bufs=2, space="PSUM") as pa,
            tc.tile_pool(name="pm", bufs=2, space="PSUM") as pm,
        ):
            # ---- load weights (single DMA) ----
            wts_sb = const.tile([S, WTS], f32)
            nc.sync.dma_start(out=wts_sb[:], in_=wts[:])
            o = 0
            prod_sb = wts_sb[:, o : o + PS]; o += PS
            pers_sb = wts_sb[:, o : o + Q]; o += Q
            w1a_sb = wts_sb[:, o : o + S]; o += S
            w1b_sb = wts_sb[:, o : o + S]; o += S
            w2e_sb = wts_sb[:, o : o + (GW * GW) // 2].bitcast(bf16)  # [S, 1024]

            x_sb = []
            for b in range(B):
                xb = xbuf.tile([PS, Q], f32, tag=f"x{b}")
                nc.sync.dma_start(out=xb[:], in_=x_in[b])
                x_sb.append(xb)

            # ---- pr_T / pe_T + exp ----
            epr = const.tile([S, PS], f32)   # exp(pr_T) [s, p]
            epe = const.tile([S, Q], f32)    # exp(pe_T) [s, q]

            ps_pr = pa.tile([S, Q // 2], f32, tag="pe")
            nc.tensor.matmul(
                out=ps_pr[:, :PS], lhsT=w1a_sb, rhs=prod_sb
            )
            nc.scalar.activation(out=epr[:], in_=ps_pr[:, :PS], func=AF.Exp)

            for h in range(2):
                ps_pe = pa.tile([S, Q // 2], f32, tag="pe")
                nc.tensor.matmul(
                    out=ps_pe[:],
                    lhsT=w1b_sb,
                    rhs=pers_sb[:, h * (Q // 2) : (h + 1) * (Q // 2)],
                )
                nc.scalar.activation(
                    out=epe[:, h * (Q // 2) : (h + 1) * (Q // 2)],
                    in_=ps_pe[:],
                    func=AF.Exp,
                )

            # ---- main loop ----
            # One shared PSUM tile: the four p-groups write disjoint
            # partition ranges of the same two banks, so no WAW deps and
            # every matmul carries at most one (ACT) semaphore wait.
            adj = const.tile([PS, Q], f32)
            m_ps = pm.tile([PS, Q], f32)
            for g in range(GROUPS):
                gsl = slice(GW * g, GW * (g + 1))
                for j in range(GW):
                    p = GW * g + j
                    h_t = hbuf.tile([S, Q], bf16, tag="h")
                    # h_p = ln(1 + Epe * Epr[:, p])  == softplus(pr_p + pe)
                    nc.scalar.activation(
                        out=h_t[:],
                        in_=epe[:],
                        func=AF.Ln,
                        bias=1.0,
                        scale=epr[:, p : p + 1],
                    )
                    for hh in range(2):
                        qsl = slice(hh * (Q // 2), (hh + 1) * (Q // 2))
                        nc.tensor.matmul(
                            out=m_ps[gsl, qsl],
                            lhsT=w2e_sb[:, j * GW : (j + 1) * GW],
                            rhs=h_t[:, qsl],
                            start=(j == 0),
                            stop=(j == GW - 1),
                            tile_position=(0, GW * g),
                        )
            # leaky relu evacuation on DVE: adj = max(m, 0.1*m). Two ops since
            # a DVE op may read only one PSUM operand. Runs after the whole
            # loop (needs all groups anyway; PE is done with these banks).
            tmp = const.tile([PS, Q], f32)
            for g in range(GROUPS):
                gsl = slice(GW * g, GW * (g + 1))
                nc.vector.tensor_scalar_mul(tmp[gsl, :], m_ps[gsl, :], 0.1)
                nc.vector.tensor_tensor(
                    out=adj[gsl, :], in0=m_ps[gsl, :], in1=tmp[gsl, :], op=ALU.max
                )

            # ---- epilogue: out[b] = adj * x[b] ----
            for b in range(B):
                ob = xbuf.tile([PS, Q], f32, tag=f"o{b}")
                nc.vector.tensor_mul(out=ob[:], in0=x_sb[b][:], in1=adj[:])
                nc.sync.dma_start(out=out_e[b], in_=ob[:])

    _strip_self_waits(nc)
    return nc


_ENGINE_SEM_PREFIX = {
    "EngineType.PE": "PE_",
    "EngineType.Activation": "Activation_",
    "EngineType.DVE": "DVE_",
    "EngineType.Pool": "Pool_",
    "EngineType.SP": "SP_sequencer_",
}


def _strip_self_waits(nc):
    """Remove semaphore waits on an instruction's own engine semaphore.

    Engines execute their instruction streams strictly in order, so a wait
    on the engine's own completion semaphore (emitted by Tile's
    non-transitive vector clock for same-engine WAW/WAR deps) is always
    already satisfied. neuronx-cc's walrus allows only ONE sync wait per
    engine instruction, so these redundant waits break codegen.
    """
    from concourse import mybir

    for f in nc.m.functions:
        for bb in f.blocks:
            for ins in bb.instructions:
                si = ins.sync_info
                if si is None or not si.on_wait:
                    continue
                drop = set()
                pref = _ENGINE_SEM_PREFIX.get(str(getattr(ins, "engine", "")))
                if pref is not None:
                    drop.update(
                        w.ant_name
                        for w in si.on_wait
                        if (w.ant_name or "").startswith(pref)
                    )
                if str(ins.opcode) == "DMACopy":
                    # Same-queue ordering waits (sem also in on_update) are
                    # redundant: none of our DMAs have data deps on each
                    # other and queue slot usage is tiny.
                    upd = {u.ant_name for u in (si.on_update or [])}
                    drop.update(
                        w.ant_name for w in si.on_wait if w.ant_name in upd
                    )
                if drop:
                    kept = [w for w in si.on_wait if w.ant_name not in drop]
                    ins.sync_info = mybir.SyncInfo(
                        on_wait=kept, on_update=list(si.on_update or [])
                    )

    # Split any instruction still carrying >1 wait: hoist extra waits onto
    # same-engine NoOps placed right before it (the TRN2 ISA has one wait
    # slot per instruction; waits execute sequentially on the sequencer).
    for f in nc.m.functions:
        for bb in f.blocks:
            out = []
            for ins in bb.instructions:
                si = ins.sync_info
                if si is not None and si.on_wait and len(si.on_wait) > 1:
                    waits = list(si.on_wait)
                    for k, w in enumerate(waits[:-1]):
                        nop = mybir.InstNoOp(name=f"{ins.name}-hw{k}", ins=[], outs=[])
                        nop.engine = ins.engine
                        nop.sync_info = mybir.SyncInfo(on_wait=[w], on_update=[])
                        out.append(nop)
                    ins.sync_info = mybir.SyncInfo(
                        on_wait=[waits[-1]], on_update=list(si.on_update or [])
                    )
                out.append(ins)
            bb.instructions = out


def _get_nc():
    if "nc" not in _CACHE:
        _CACHE["nc"] = _build_nc()
    return _CACHE["nc"]


def make_in_maps(x, product, person, w1, w2):
    import ml_dtypes

    x = np.ascontiguousarray(np.asarray(x, dtype=np.float32))
    product = np.asarray(product, dtype=np.float32)
    person = np.asarray(person, dtype=np.float32)
    w1 = np.asarray(w1, dtype=np.float32)
    w2 = np.asarray(w2, dtype=np.float32)

    pers_t = np.ascontiguousarray(person.T)           # [S, Q]
    w1a = np.ascontiguousarray(w1[:S])                # [S, S]
    w1b = np.ascontiguousarray(w1[S:])                # [S, S]

    # column-embedded w2: w2e[k, j, m] = w2[k] if m == j else 0
    w2e = np.zeros((S, GW, GW), dtype=np.float32)
    idx = np.arange(GW)
    w2e[:, idx, idx] = w2[:, 0][:, None]
    w2e_f32view = (
        w2e.astype(ml_dtypes.bfloat16)
        .reshape(S, GW * GW)
        .view(np.float32)  # [S, 512]: bf16 pairs packed in f32 words
    )

    in_maps = []
    for i in range(N_CORES):
        sl = slice(PS * i, PS * (i + 1))
        wts = np.concatenate(
            [np.ascontiguousarray(product[sl].T), pers_t, w1a, w1b, w2e_f32view],
            axis=1,
        )
        in_maps.append(
            {
                "wts": np.ascontiguousarray(wts),       # [S, WTS]
                "x": np.ascontiguousarray(x[:, sl, :]),  # [B, PS, Q]
            }
        )
    return in_maps


def run(x, product, person, w1, w2, trace=False, **kw):
    from concourse.bass_utils import run_bass_kernel_spmd

    nc = _get_nc()
    in_maps = make_in_maps(x, product, person, w1, w2)
    res = run_bass_kernel_spmd(
        nc, in_maps, core_ids=list(range(N_CORES)), trace=trace, **kw
    )
    outs = [np.asarray(r["out"]) for r in res.results]
    full = np.concatenate(outs, axis=1).astype(np.float32)
    return full, res


def kernel(x, product, person, w1, w2):
    full, _ = run(x, product, person, w1, w2, trace=False)
    return full


# revision 18
# speedup vs baseline: 1.0799x; 1.0799x over previous
"""Trainium2 Bass kernel for nn_Adjacency (dense_mlp).

Reference computation:
    pr = product @ w1[:S]                # [P, S]
    pe = person  @ w1[S:]                # [Q, S]
    h  = softplus(pr[:,None,:] + pe[None,:,:])   # [P, Q, S]
    m  = einsum('pqs,so->pq', h, w2)
    adj = leaky_relu(m, 0.1)
    out = adj[None] * x                  # [B, P, Q]

Sharding: P across 8 cores (128 rows each); person/w1/w2 replicated;
x / out sharded on dim 1. No collectives.

Per-core algorithm (all tiles [partition, free]):
  - pr_T/pe_T via TensorE matmuls (host passes pre-transposed operands);
    Epr = exp(pr_T) [s,p], Epe = exp(pe_T) [s,q] on ACT.
  - for p in 0..127: h_p[s, q] = ln(Epe * Epr[:, p] + 1) — ONE ACT
    instruction per p (softplus(pr+pe) = ln(1 + exp(pr)exp(pe)); the
    per-partition `scale` AP fuses the broadcast multiply). The hardware
    has no softplus table; exp+ln share one table set. This Ln stream is
    the kernel's critical path (~141 us).
  - S-reduction on TensorE: m[p, :] = w2^T @ h_p via M=32 matmuls with
    "column-embedded" w2 (lhsT_j has w2 in column j, zeros elsewhere);
    32 accumulating matmuls build a dense [32 p, 512 q] PSUM tile, and
    the four p-groups land on the four 32-aligned PSUM col-groups.
  - per-group epilogue pipelined under the Ln stream: leaky-relu
    evacuation on DVE, bf16 multiplies with x, DMA out.
"""

import numpy as np

P, Q, S, B = 1024, 1024, 128, 8
N_CORES = 8
PS = P // N_CORES  # 128 p rows per core
GROUPS = 4         # p-groups of 32 (PSUM col-groups)
GW = PS // GROUPS  # 32

_CACHE = {}


def _build_nc():
    import concourse.bass as bass
    import concourse.tile as tile
    from concourse import mybir

    f32 = mybir.dt.float32
    bf16 = mybir.dt.bfloat16
    AF = mybir.ActivationFunctionType
    ALU = mybir.AluOpType

    nc = bass.Bass()

    # Two weight blobs (few DMAs -> few semaphore waits; walrus allows only
    # one sync wait per instruction). Blob A feeds the pe path (needed
    # first), blob B the pr path + embedded w2.
    WA = Q + S                       # person_t | w1b
    WB = PS + S + (GW * GW) // 2     # product_t | w1a | w2emb(bf16 packed)
    wa = nc.declare_dram_parameter("wa", [S, WA], f32, isOutput=False)
    wb = nc.declare_dram_parameter("wb", [S, WB], f32, isOutput=False)
    x_in = nc.declare_dram_parameter("x", [B, PS, Q], bf16, isOutput=False)
    out_e = nc.declare_dram_parameter("out", [B, PS, Q], bf16, isOutput=True)

    with tile.TileContext(nc) as tc:
        with (
            tc.tile_pool(name="const", bufs=1) as const,
            tc.tile_pool(name="xbuf", bufs=1) as xbuf,
            tc.tile_pool(name="hbuf", bufs=4) as hbuf,
            tc.tile_pool(name="pa", bufs=2, space="PSUM") as pa,
            tc.tile_pool(name="pm", bufs=2, space="PSUM") as pm,
        ):
            # Preload the exp/ln ACT table set while DMAs run: a dummy exp
            # on a memset tile has no input deps.
            scratch = const.tile([S, 1], f32)
            nc.vector.memset(scratch[:], 0.0)
            nc.scalar.activation(out=scratch[:], in_=scratch[:], func=AF.Exp)

            # ---- load weights ----
            wa_sb = const.tile([S, WA], f32)
            wb_sb = const.tile([S, WB], f32)
            nc.sync.dma_start(out=wa_sb[:], in_=wa[:])
            nc.sync.dma_start(out=wb_sb[:], in_=wb[:])
            pers_sb = wa_sb[:, 0:Q]
            w1b_sb = wa_sb[:, Q : Q + S]
            prod_sb = wb_sb[:, 0:PS]
            w1a_sb = wb_sb[:, PS : PS + S]
            w2e_sb = wb_sb[:, PS + S : PS + S + (GW * GW) // 2].bitcast(bf16)

            x_sb = []
            for b in range(B):
                xb = xbuf.tile([PS, Q], bf16, tag=f"x{b}")
                nc.sync.dma_start(out=xb[:], in_=x_in[b])
                x_sb.append(xb)

            # ---- pr_T / pe_T + exp ----
            epr = const.tile([S, PS], f32)   # exp(pr_T) [s, p]
            epe = const.tile([S, Q], f32)    # exp(pe_T) [s, q]

            for h in range(2):
                ps_pe = pa.tile([S, Q // 2], f32, tag="pe")
                nc.tensor.matmul(
                    out=ps_pe[:],
                    lhsT=w1b_sb,
                    rhs=pers_sb[:, h * (Q // 2) : (h + 1) * (Q // 2)],
                )
                nc.scalar.activation(
                    out=epe[:, h * (Q // 2) : (h + 1) * (Q // 2)],
                    in_=ps_pe[:],
                    func=AF.Exp,
                )
            ps_pr = pa.tile([S, Q // 2], f32, tag="pe")
            nc.tensor.matmul(out=ps_pr[:, :PS], lhsT=w1a_sb, rhs=prod_sb)
            nc.scalar.activation(out=epr[:], in_=ps_pr[:, :PS], func=AF.Exp)

            # ---- main loop with per-group pipelined epilogue ----
            adj = const.tile([PS, Q], bf16)
            tmp = const.tile([PS, Q], bf16)
            ob = []
            for b in range(B):
                obt = xbuf.tile([PS, Q], bf16, tag=f"o{b}")
                ob.append(obt)
            for g in range(GROUPS):
                m_ps = pm.tile([PS, Q], f32)  # rows [32g, 32g+32) used
                gsl = slice(GW * g, GW * (g + 1))
                for j in range(GW):
                    p = GW * g + j
                    h_t = hbuf.tile([S, Q], bf16, tag="h")
                    # h_p = ln(1 + Epe * Epr[:, p])  == softplus(pr_p + pe)
                    nc.scalar.activation(
                        out=h_t[:],
                        in_=epe[:],
                        func=AF.Ln,
                        bias=1.0,
                        scale=epr[:, p : p + 1],
                    )
                    for hh in range(2):
                        qsl = slice(hh * (Q // 2), (hh + 1) * (Q // 2))
                        nc.tensor.matmul(
                            out=m_ps[gsl, qsl],
                            lhsT=w2e_sb[:, j * GW : (j + 1) * GW],
                            rhs=h_t[:, qsl],
                            start=(j == 0),
                            stop=(j == GW - 1),
                            tile_position=(0, GW * g),
                        )
                # leaky relu evacuation on DVE: adj = max(m, 0.1*m)
                # (a DVE op may read only one PSUM operand -> two steps)
                nc.vector.tensor_scalar_mul(tmp[gsl, :], m_ps[gsl, :], 0.1)
                nc.vector.tensor_tensor(
                    out=adj[gsl, :], in0=m_ps[gsl, :], in1=tmp[gsl, :], op=ALU.max
                )
                # out[b] = adj * x[b] for this group's rows (bf16, 2x/4x DVE)
                for b in range(B):
                    nc.vector.tensor_mul(
                        out=ob[b][gsl, :], in0=x_sb[b][gsl, :], in1=adj[gsl, :]
                    )
                    nc.sync.dma_start(out=out_e[b, gsl, :], in_=ob[b][gsl, :])

    _fix_waits(nc)
    return nc


_ENGINE_SEM_PREFIX = {
    "EngineType.PE": "PE_",
    "EngineType.Activation": "Activation_",
    "EngineType.DVE": "DVE_",
    "EngineType.Pool": "Pool_",
    "EngineType.SP": "SP_sequencer_",
}


def _fix_waits(nc):
    """Make every instruction carry at most ONE semaphore wait (the TRN2
    ISA / neuronx-cc walrus limit).

    1. Strip waits on an instruction's own engine semaphore: engines
       execute strictly in order, so same-engine WAW/WAR waits (emitted by
       Tile's non-transitive vector clock) are always already satisfied.
    2. Strip same-queue ordering waits on DMAs (sem also in on_update):
       hardware DMA queues are FIFO and none of our DMAs have data deps on
       each other.
    3. Hoist any remaining extra waits onto same-engine NoOps inserted
       right before the instruction (waits execute sequentially on the
       sequencer).
    """
    from concourse import mybir

    for f in nc.m.functions:
        for bb in f.blocks:
            for ins in bb.instructions:
                si = ins.sync_info
                if si is None or not si.on_wait:
                    continue
                drop = set()
                pref = _ENGINE_SEM_PREFIX.get(str(getattr(ins, "engine", "")))
                if pref is not None:
                    drop.update(
                        w.ant_name
                        for w in si.on_wait
                        if (w.ant_name or "").startswith(pref)
                    )
                if str(ins.opcode) == "DMACopy":
                    upd = {u.ant_name for u in (si.on_update or [])}
                    drop.update(w.ant_name for w in si.on_wait if w.ant_name in upd)
                if drop:
                    kept = [w for w in si.on_wait if w.ant_name not in drop]
                    ins.sync_info = mybir.SyncInfo(
                        on_wait=kept, on_update=list(si.on_update or [])
                    )

    for f in nc.m.functions:
        for bb in f.blocks:
            out = []
            for ins in bb.instructions:
                si = ins.sync_info
                if si is not None and si.on_wait and len(si.on_wait) > 1:
                    waits = list(si.on_wait)
                    for k, w in enumerate(waits[:-1]):
                        nop = mybir.InstNoOp(name=f"{ins.name}-hw{k}", ins=[], outs=[])
                        nop.engine = ins.engine
                        nop.sync_info = mybir.SyncInfo(on_wait=[w], on_update=[])
                        out.append(nop)
                    ins.sync_info = mybir.SyncInfo(
                        on_wait=[waits[-1]], on_update=list(si.on_update or [])
                    )
                out.append(ins)
            bb.instructions = out


def _get_nc():
    if "nc" not in _CACHE:
        _CACHE["nc"] = _build_nc()
    return _CACHE["nc"]


def make_in_maps(x, product, person, w1, w2):
    import ml_dtypes

    bf16 = ml_dtypes.bfloat16
    x = np.asarray(x, dtype=np.float32)
    product = np.asarray(product, dtype=np.float32)
    person = np.asarray(person, dtype=np.float32)
    w1 = np.asarray(w1, dtype=np.float32)
    w2 = np.asarray(w2, dtype=np.float32)

    pers_t = np.ascontiguousarray(person.T)           # [S, Q]
    w1a = np.ascontiguousarray(w1[:S])                # [S, S]
    w1b = np.ascontiguousarray(w1[S:])                # [S, S]

    # column-embedded w2: w2e[k, j, m] = w2[k] if m == j else 0
    w2e = np.zeros((S, GW, GW), dtype=np.float32)
    idx = np.arange(GW)
    w2e[:, idx, idx] = w2[:, 0][:, None]
    w2e_f32view = (
        w2e.astype(bf16).reshape(S, GW * GW).view(np.float32)  # [S, 512]
    )

    wa = np.ascontiguousarray(np.concatenate([pers_t, w1b], axis=1))
    x_bf = x.astype(bf16)

    in_maps = []
    for i in range(N_CORES):
        sl = slice(PS * i, PS * (i + 1))
        wb = np.concatenate(
            [np.ascontiguousarray(product[sl].T), w1a, w2e_f32view], axis=1
        )
        in_maps.append(
            {
                "wa": wa,
                "wb": np.ascontiguousarray(wb),
                "x": np.ascontiguousarray(x_bf[:, sl, :]),
            }
        )
    return in_maps


def run(x, product, person, w1, w2, trace=False, **kw):
    from concourse.bass_utils import run_bass_kernel_spmd

    nc = _get_nc()
    in_maps = make_in_maps(x, product, person, w1, w2)
    res = run_bass_kernel_spmd(
        nc, in_maps, core_ids=list(range(N_CORES)), trace=trace, **kw
    )
    outs = [np.asarray(r["out"]).astype(np.float32) for r in res.results]
    full = np.concatenate(outs, axis=1)
    return full, res


def kernel(x, product, person, w1, w2):
    full, _ = run(x, product, person, w1, w2, trace=False)
    return full


# revision 23
# speedup vs baseline: 1.0882x; 1.0077x over previous
"""Trainium2 Bass kernel for nn_Adjacency (dense_mlp).

Reference computation:
    pr = product @ w1[:S]                # [P, S]
    pe = person  @ w1[S:]                # [Q, S]
    h  = softplus(pr[:,None,:] + pe[None,:,:])   # [P, Q, S]
    m  = einsum('pqs,so->pq', h, w2)
    adj = leaky_relu(m, 0.1)
    out = adj[None] * x                  # [B, P, Q]

Sharding: P across 8 cores (128 rows each); person/w1/w2 replicated;
x / out sharded on dim 1. No collectives.

Per-core algorithm (all tiles [partition, free]):
  - pr_T/pe_T via TensorE matmuls (host passes pre-transposed operands);
    Epr = exp(pr_T) [s,p], Epe = exp(pe_T) [s,q] on ACT.
  - for p in 0..127: h_p[s, q] = ln(Epe * Epr[:, p] + 1) — ONE ACT
    instruction per p (softplus(pr+pe) = ln(1 + exp(pr)exp(pe)); the
    per-partition `scale` AP fuses the broadcast multiply). The hardware
    has no softplus table; exp+ln share one table set. This Ln stream is
    the kernel's critical path (~141 us).
  - S-reduction on TensorE: m[p, :] = w2^T @ h_p via M=32 matmuls with
    "column-embedded" w2 (lhsT_j has w2 in column j, zeros elsewhere);
    32 accumulating matmuls build a dense [32 p, 512 q] PSUM tile, and
    the four p-groups land on the four 32-aligned PSUM col-groups.
  - per-group epilogue pipelined under the Ln stream: leaky-relu
    evacuation on DVE, bf16 multiplies with x, DMA out.
"""

import numpy as np

P, Q, S, B = 1024, 1024, 128, 8
N_CORES = 8
PS = P // N_CORES  # 128 p rows per core
GROUPS = 4         # p-groups of 32 (PSUM col-groups)
GW = PS // GROUPS  # 32

_CACHE = {}


def _build_nc():
    import concourse.bass as bass
    import concourse.tile as tile
    from concourse import mybir

    f32 = mybir.dt.float32
    bf16 = mybir.dt.bfloat16
    AF = mybir.ActivationFunctionType
    ALU = mybir.AluOpType

    nc = bass.Bass()

    # Two weight blobs (few DMAs -> few semaphore waits; walrus allows only
    # one sync wait per instruction). Blob A feeds the pe path (needed
    # first), blob B the pr path + embedded w2.
    WA = Q + S                       # person_t | w1b
    WB = PS + S + (GW * GW) // 2     # product_t | w1a | w2emb(bf16 packed)
    wa = nc.declare_dram_parameter("wa", [S, WA], f32, isOutput=False)
    wb = nc.declare_dram_parameter("wb", [S, WB], f32, isOutput=False)
    x_in = nc.declare_dram_parameter("x", [B, PS, Q], bf16, isOutput=False)
    out_e = nc.declare_dram_parameter("out", [B, PS, Q], bf16, isOutput=True)

    with tile.TileContext(nc) as tc:
        with (
            tc.tile_pool(name="const", bufs=1) as const,
            tc.tile_pool(name="xbuf", bufs=1) as xbuf,
            tc.tile_pool(name="hbuf", bufs=4) as hbuf,
            tc.tile_pool(name="pa", bufs=2, space="PSUM") as pa,
            tc.tile_pool(name="pm", bufs=2, space="PSUM") as pm,
        ):
            # Preload the exp/ln ACT table set while DMAs run: a dummy exp
            # on a memset tile has no input deps.
            scratch = const.tile([S, 1], f32)
            nc.vector.memset(scratch[:], 0.0)
            nc.scalar.activation(out=scratch[:], in_=scratch[:], func=AF.Exp)

            # ---- load weights ----
            wa_sb = const.tile([S, WA], f32)
            wb_sb = const.tile([S, WB], f32)
            nc.sync.dma_start(out=wa_sb[:], in_=wa[:])
            nc.sync.dma_start(out=wb_sb[:], in_=wb[:])
            pers_sb = wa_sb[:, 0:Q]
            w1b_sb = wa_sb[:, Q : Q + S]
            prod_sb = wb_sb[:, 0:PS]
            w1a_sb = wb_sb[:, PS : PS + S]
            w2e_sb = wb_sb[:, PS + S : PS + S + (GW * GW) // 2].bitcast(bf16)

            # ---- pr_T / pe_T + exp ----
            epr = const.tile([S, PS], f32)   # exp(pr_T) [s, p]
            epe = const.tile([S, Q], f32)    # exp(pe_T) [s, q]

            for h in range(2):
                ps_pe = pa.tile([S, Q // 2], f32, tag="pe")
                nc.tensor.matmul(
                    out=ps_pe[:],
                    lhsT=w1b_sb,
                    rhs=pers_sb[:, h * (Q // 2) : (h + 1) * (Q // 2)],
                )
                nc.scalar.activation(
                    out=epe[:, h * (Q // 2) : (h + 1) * (Q // 2)],
                    in_=ps_pe[:],
                    func=AF.Exp,
                )
            ps_pr = pa.tile([S, Q // 2], f32, tag="pe")
            nc.tensor.matmul(out=ps_pr[:, :PS], lhsT=w1a_sb, rhs=prod_sb)
            nc.scalar.activation(out=epr[:], in_=ps_pr[:, :PS], func=AF.Exp)

            # x DMAs issued from the DVE queue, gated on epe: keeps them off
            # the HBM while the critical-path weight DMAs + setup run. They
            # are only needed ~40us in.
            from concourse.tile import add_dep_helper

            gate = const.tile([S, 1], f32)
            g_ins = nc.gpsimd.tensor_copy(out=gate[:], in_=epe[:, 0:1])
            x_sb = []
            for b in range(B):
                xb = xbuf.tile([PS, Q], bf16, tag=f"x{b}")
                d = nc.gpsimd.dma_start(out=xb[:], in_=x_in[b])
                add_dep_helper(d.ins, g_ins.ins, False, "x-dma after epe gate")
                x_sb.append(xb)

            # ---- main loop with per-group pipelined epilogue ----
            adj = const.tile([PS, Q], bf16)
            tmp = const.tile([PS, Q], bf16)
            ob = []
            for b in range(B):
                obt = xbuf.tile([PS, Q], bf16, tag=f"o{b}")
                ob.append(obt)
            for g in range(GROUPS):
                m_ps = pm.tile([PS, Q], f32)  # rows [32g, 32g+32) used
                gsl = slice(GW * g, GW * (g + 1))
                for j in range(GW):
                    p = GW * g + j
                    h_t = hbuf.tile([S, Q], bf16, tag="h")
                    # h_p = ln(1 + Epe * Epr[:, p])  == softplus(pr_p + pe)
                    nc.scalar.activation(
                        out=h_t[:],
                        in_=epe[:],
                        func=AF.Ln,
                        bias=1.0,
                        scale=epr[:, p : p + 1],
                    )
                    for hh in range(2):
                        qsl = slice(hh * (Q // 2), (hh + 1) * (Q // 2))
                        nc.tensor.matmul(
                            out=m_ps[gsl, qsl],
                            lhsT=w2e_sb[:, j * GW : (j + 1) * GW],
                            rhs=h_t[:, qsl],
                            start=(j == 0),
                            stop=(j == GW - 1),
                            tile_position=(0, GW * g),
                        )
                # leaky relu evacuation: DVE for groups hidden under the Ln
                # stream; ACT Prelu (same table set) for the last group,
                # where ACT is idle and DVE latency would be the tail.
                if g < GROUPS - 1:
                    # a DVE op may read only one PSUM operand -> two steps
                    nc.vector.tensor_scalar_mul(tmp[gsl, :], m_ps[gsl, :], 0.1)
                    nc.vector.tensor_tensor(
                        out=adj[gsl, :], in0=m_ps[gsl, :], in1=tmp[gsl, :], op=ALU.max
                    )
                else:
                    nc.scalar.activation(
                        out=adj[gsl, :], in_=m_ps[gsl, :], func=AF.Prelu, alpha=0.1
                    )
                # out[b] = adj * x[b] for this group's rows (bf16, 2x/4x DVE)
                for b in range(B):
                    nc.vector.tensor_mul(
                        out=ob[b][gsl, :], in0=x_sb[b][gsl, :], in1=adj[gsl, :]
                    )
                    nc.sync.dma_start(out=out_e[b, gsl, :], in_=ob[b][gsl, :])

    _fix_waits(nc)
    return nc


_ENGINE_SEM_PREFIX = {
    "EngineType.PE": "PE_",
    "EngineType.Activation": "Activation_",
    "EngineType.DVE": "DVE_",
    "EngineType.Pool": "Pool_",
    "EngineType.SP": "SP_sequencer_",
}


def _fix_waits(nc):
    """Make every instruction carry at most ONE semaphore wait (the TRN2
    ISA / neuronx-cc walrus limit).

    1. Strip waits on an instruction's own engine semaphore: engines
       execute strictly in order, so same-engine WAW/WAR waits (emitted by
       Tile's non-transitive vector clock) are always already satisfied.
    2. Strip same-queue ordering waits on DMAs (sem also in on_update):
       hardware DMA queues are FIFO and none of our DMAs have data deps on
       each other.
    3. Hoist any remaining extra waits onto same-engine NoOps inserted
       right before the instruction (waits execute sequentially on the
       sequencer).
    """
    from concourse import mybir

    for f in nc.m.functions:
        for bb in f.blocks:
            for ins in bb.instructions:
                si = ins.sync_info
                if si is None or not si.on_wait:
                    continue
                drop = set()
                pref = _ENGINE_SEM_PREFIX.get(str(getattr(ins, "engine", "")))
                if pref is not None:
                    drop.update(
                        w.ant_name
                        for w in si.on_wait
                        if (w.ant_name or "").startswith(pref)
                    )
                if str(ins.opcode) == "DMACopy":
                    upd = {u.ant_name for u in (si.on_update or [])}
                    drop.update(w.ant_name for w in si.on_wait if w.ant_name in upd)
                if drop:
                    kept = [w for w in si.on_wait if w.ant_name not in drop]
                    ins.sync_info = mybir.SyncInfo(
                        on_wait=kept, on_update=list(si.on_update or [])
                    )

    for f in nc.m.functions:
        for bb in f.blocks:
            out = []
            for ins in bb.instructions:
                si = ins.sync_info
                if si is not None and si.on_wait and len(si.on_wait) > 1:
                    waits = list(si.on_wait)
                    for k, w in enumerate(waits[:-1]):
                        nop = mybir.InstNoOp(name=f"{ins.name}-hw{k}", ins=[], outs=[])
                        nop.engine = ins.engine
                        nop.sync_info = mybir.SyncInfo(on_wait=[w], on_update=[])
                        out.append(nop)
                    ins.sync_info = mybir.SyncInfo(
                        on_wait=[waits[-1]], on_update=list(si.on_update or [])
                    )
                out.append(ins)
            bb.instructions = out


def _get_nc():
    if "nc" not in _CACHE:
        _CACHE["nc"] = _build_nc()
    return _CACHE["nc"]


def make_in_maps(x, product, person, w1, w2):
    import ml_dtypes

    bf16 = ml_dtypes.bfloat16
    x = np.asarray(x, dtype=np.float32)
    product = np.asarray(product, dtype=np.float32)
    person = np.asarray(person, dtype=np.float32)
    w1 = np.asarray(w1, dtype=np.float32)
    w2 = np.asarray(w2, dtype=np.float32)

    pers_t = np.ascontiguousarray(person.T)           # [S, Q]
    w1a = np.ascontiguousarray(w1[:S])                # [S, S]
    w1b = np.ascontiguousarray(w1[S:])                # [S, S]

    # column-embedded w2: w2e[k, j, m] = w2[k] if m == j else 0
    w2e = np.zeros((S, GW, GW), dtype=np.float32)
    idx = np.arange(GW)
    w2e[:, idx, idx] = w2[:, 0][:, None]
    w2e_f32view = (
        w2e.astype(bf16).reshape(S, GW * GW).view(np.float32)  # [S, 512]
    )

    wa = np.ascontiguousarray(np.concatenate([pers_t, w1b], axis=1))
    x_bf = x.astype(bf16)

    in_maps = []
    for i in range(N_CORES):
        sl = slice(PS * i, PS * (i + 1))
        wb = np.concatenate(
            [np.ascontiguousarray(product[sl].T), w1a, w2e_f32view], axis=1
        )
        in_maps.append(
            {
                "wa": wa,
                "wb": np.ascontiguousarray(wb),
                "x": np.ascontiguousarray(x_bf[:, sl, :]),
            }
        )
    return in_maps


def run(x, product, person, w1, w2, trace=False, **kw):
    from concourse.bass_utils import run_bass_kernel_spmd

    nc = _get_nc()
    in_maps = make_in_maps(x, product, person, w1, w2)
    res = run_bass_kernel_spmd(
        nc, in_maps, core_ids=list(range(N_CORES)), trace=trace, **kw
    )
    outs = [np.asarray(r["out"]).astype(np.float32) for r in res.results]
    full = np.concatenate(outs, axis=1)
    return full, res


def kernel(x, product, person, w1, w2):
    full, _ = run(x, product, person, w1, w2, trace=False)
    return full
